# revision 5
# baseline (speedup 1.0000x reference)
"""Trainium2 Bass kernel for a 2-layer GCN decoder (nn_GCNDecoder).

Strategy (8 NeuronCores, SPMD):
  - Destination nodes sharded 8 ways. Edges (with self-loops) partitioned by
    dst shard, grouped by dst into blocks of <=32 distinct dsts (W) x <=512
    edge lanes (CPB=4 chunks of 128). 8 blocks form a "group" = 256 PSUM
    columns processed by one weight-chain pass.
  - GCN normalization norm_e = dinv[src]*dinv[dst] is folded into the staged
    messages on the host (msg = x[src]*norm). The per-chunk selection matrix
    S[lane, slot] = (iota==slot_id) is built on the DVE with ONE broadcast
    scalar_tensor_tensor op per group; segment-sum aggregation is a PE matmul
    agg[Cin, slot] += M^T S with PSUM accumulation.
  - Layer 1 fuses BOTH weight matmuls transposed (constant stationary):
    z^T = W4^T relu(W3^T agg + b3), so layer-2 messages are 64-wide.
    Layer 2 is aggregation + b4 only.
  - Source features are staged per-edge-lane by the host (halo exchange):
    layer 1 messages from x*norm, layer 2 messages from z*norm, where z is
    re-staged between the two device programs.

The host does: integer packing/sorting, degree->norm prep, staging of
per-lane message rows, and output unpermutation.
"""

import os
import sys
import numpy as np
import ml_dtypes

bf16 = ml_dtypes.bfloat16

# problem constants (spec: nn_GCNDecoder_32959579030036)
N_NODES = 100000
IN_C = 64
HID_C = 128
OUT_C = 64
N_CORES = 8
SHARD = N_NODES // N_CORES   # 12500

W = 32                        # dst slots per block
CPB = 4                       # chunks (of 128 edge lanes) per block
SLOTS = CPB * 128             # 512 edge lanes per block
GRP = 8                       # blocks per group (8*W = 256 psum columns)
GCOLS = GRP * W               # 256
GCH = GRP * CPB               # 32 chunks per group
SG = 8                        # groups per output stage chunk / msg DMA batch

_BASS_READY = False


def _import_bass():
    global _BASS_READY, bacc, tile, mybir, bass_utils
    if _BASS_READY:
        return
    for p in ("/opt/trn_rl_repo", "/opt/pypackages"):
        if os.path.isdir(p) and p not in sys.path:
            sys.path.append(p)
    import concourse.bacc as bacc
    import concourse.tile as tile
    import concourse.mybir as mybir
    from concourse import bass_utils
    _BASS_READY = True


# ----------------------------------------------------------------------------
# host-side packing
# ----------------------------------------------------------------------------

def _pack_core(src, dst, norm):
    order = np.argsort(dst, kind="stable")
    src, dst, norm = src[order], dst[order], norm[order]
    uniq, seg_start = np.unique(dst, return_index=True)
    seg_end = np.append(seg_start[1:], len(dst))
    seg_len = seg_end - seg_start
    assert seg_len.max() <= SLOTS, "node in-degree exceeds block capacity"

    blocks, cur, cur_slots = [], [], 0
    for i in range(len(uniq)):
        if cur and (cur_slots + seg_len[i] > SLOTS or len(cur) >= W):
            blocks.append(cur)
            cur, cur_slots = [], 0
        cur.append(i)
        cur_slots += seg_len[i]
    if cur:
        blocks.append(cur)

    nb = len(blocks)
    e_src = np.zeros((nb, SLOTS), np.int64)
    e_slot = np.zeros((nb, SLOTS), np.float32)
    e_norm = np.zeros((nb, SLOTS), np.float32)
    slot_node = np.full((nb, W), -1, np.int64)
    for b, segs in enumerate(blocks):
        ps, pl, pn = [], [], []
        for s_local, i in enumerate(segs):
            sl = slice(seg_start[i], seg_end[i])
            ps.append(src[sl])
            pl.append(np.full(seg_len[i], s_local, np.float32))
            pn.append(norm[sl])
            slot_node[b, s_local] = uniq[i]
        bs, bslot, bn = map(np.concatenate, (ps, pl, pn))
        o = np.argsort(bs, kind="stable")
        n = len(bs)
        e_src[b, :n] = bs[o]
        e_slot[b, :n] = bslot[o]
        e_norm[b, :n] = bn[o]
    return dict(nb=nb, e_src=e_src, e_slot=e_slot, e_norm=e_norm,
                slot_node=slot_node)


def preprocess(x, edge_index):
    src = np.asarray(edge_index[0], np.int64)
    dst = np.asarray(edge_index[1], np.int64)
    loops = np.arange(N_NODES, dtype=np.int64)
    src_all = np.concatenate([src, loops])
    dst_all = np.concatenate([dst, loops])
    deg = np.bincount(dst_all, minlength=N_NODES).astype(np.float32)
    dinv = 1.0 / np.sqrt(deg)
    norm_all = (dinv[src_all] * dinv[dst_all]).astype(np.float32)

    shard_of = dst_all // SHARD
    cores = []
    for c in range(N_CORES):
        m = shard_of == c
        cores.append(_pack_core(src_all[m], dst_all[m], norm_all[m]))

    NB = max(c["nb"] for c in cores)
    NB = ((NB + GRP - 1) // GRP) * GRP       # pad to whole groups

    for c in cores:
        pad = NB - c["nb"]
        if pad:
            c["e_src"] = np.concatenate([c["e_src"], np.zeros((pad, SLOTS), np.int64)])
            c["e_slot"] = np.concatenate([c["e_slot"], np.zeros((pad, SLOTS), np.float32)])
            c["e_norm"] = np.concatenate([c["e_norm"], np.zeros((pad, SLOTS), np.float32)])
            c["slot_node"] = np.concatenate([c["slot_node"], np.full((pad, W), -1, np.int64)])

    # stage_row[node] = flat column index of that node's slot in the
    # concatenated [cores x NB*W] transposed stage
    stage_row = np.full(N_NODES, -1, np.int64)
    for ci, c in enumerate(cores):
        sn = c["slot_node"].ravel()
        valid = sn >= 0
        stage_row[sn[valid]] = ci * NB * W + np.nonzero(valid)[0]
    assert (stage_row >= 0).all()

    NCH = NB * CPB
    out = dict(NB=NB, NCH=NCH, stage_row=stage_row, cores=[])
    for c in cores:
        e_src = c["e_src"].reshape(NCH, 128)
        meta_slot = np.ascontiguousarray(
            c["e_slot"].reshape(NCH, 128).T).astype(bf16)   # [128,NCH]
        e_norm = np.ascontiguousarray(c["e_norm"].reshape(NCH, 128).T)
        g2 = stage_row[e_src]                                # [NCH,128]
        g2_ind = np.ascontiguousarray(g2.T)                  # [128,NCH]
        src_t = np.ascontiguousarray(e_src.T)                # [128,NCH]
        out["cores"].append(dict(meta_slot=meta_slot, e_norm=e_norm,
                                 g2_ind=g2_ind, src_t=src_t))
    return out


def _stage_msgs(rows, ind, norm):
    """rows [N, C] f32/bf16; ind [128, NCH]; norm [128, NCH] f32
    -> [128, NCH*C] bf16 messages with norm folded in."""
    C = rows.shape[1]
    m = rows[ind].astype(np.float32)          # [128, NCH, C]
    m *= norm[:, :, None]
    return np.ascontiguousarray(m.astype(bf16).reshape(128, -1))


# ----------------------------------------------------------------------------
# device programs
# ----------------------------------------------------------------------------

def build_layer1(NB, reps=1, loop_reps=0):
    """agg(x*norm) -> z^T = W4^T relu(W3^T agg + b3); z staged transposed.

    Inputs:  msg [128, NCH*IN_C] bf16, slot [128, NCH] bf16,
             iota [128, W] bf16, w3 [IN_C, HID_C] bf16, b3c [HID_C,1] f32,
             w4 [HID_C, OUT_C] bf16
    Output:  zstage [OUT_C, NB*W] bf16   (transposed: feature-major)
    """
    _import_bass()
    NCH = NB * CPB
    NG = NB // GRP
    COLS = NB * W

    nc = bacc.Bacc("TRN2", target_bir_lowering=False, debug=False,
                   num_devices=N_CORES)
    msg_d = nc.dram_tensor("msg", [128, NCH * IN_C], mybir.dt.bfloat16,
                           kind="ExternalInput")
    slot_d = nc.dram_tensor("slot", [128, NCH], mybir.dt.bfloat16,
                            kind="ExternalInput")
    iota_d = nc.dram_tensor("iota", [128, W], mybir.dt.bfloat16,
                            kind="ExternalInput")
    w3_d = nc.dram_tensor("w3", [IN_C, HID_C], mybir.dt.bfloat16,
                          kind="ExternalInput")
    b3_d = nc.dram_tensor("b3c", [HID_C, 1], mybir.dt.float32,
                          kind="ExternalInput")
    w4_d = nc.dram_tensor("w4", [HID_C, OUT_C], mybir.dt.bfloat16,
                          kind="ExternalInput")
    zst_d = nc.dram_tensor("zstage", [OUT_C, COLS], mybir.dt.bfloat16,
                           kind="ExternalOutput")

    Relu = mybir.ActivationFunctionType.Relu
    Copy = mybir.ActivationFunctionType.Copy

    with tile.TileContext(nc) as tc:
        with (
            tc.tile_pool(name="const", bufs=1) as constp,
            tc.tile_pool(name="msgs", bufs=3) as msgp,
            tc.tile_pool(name="stg", bufs=2) as stgp,
            tc.tile_pool(name="sbuf", bufs=3) as sb,
            tc.tile_pool(name="stmp", bufs=3) as stp,
            tc.tile_pool(name="pagg", bufs=2, space="PSUM") as pagg,
            tc.tile_pool(name="ph", bufs=2, space="PSUM") as ph,
            tc.tile_pool(name="pz", bufs=2, space="PSUM") as pz,
        ):
            iota_t = constp.tile([128, W], mybir.dt.bfloat16)
            nc.sync.dma_start(iota_t[:], iota_d.ap())
            slot_t = constp.tile([128, NCH], mybir.dt.bfloat16)
            nc.sync.dma_start(slot_t[:], slot_d.ap())
            w3_t = constp.tile([IN_C, HID_C], mybir.dt.bfloat16)
            nc.sync.dma_start(w3_t[:], w3_d.ap())
            b3_t = constp.tile([HID_C, 1], mybir.dt.float32)
            nc.sync.dma_start(b3_t[:], b3_d.ap())
            w4_t = constp.tile([HID_C, OUT_C], mybir.dt.bfloat16)
            nc.sync.dma_start(w4_t[:], w4_d.ap())

            def body():
                for g0 in range(0, NG, SG):
                    ngr = min(SG, NG - g0)
                    # message DMA for SG groups at once
                    k0 = g0 * GCH
                    nch = ngr * GCH
                    mt = msgp.tile([128, SG * GCH * IN_C], mybir.dt.bfloat16,
                                   tag="mt")
                    nc.sync.dma_start(
                        mt[:, :nch * IN_C],
                        msg_d.ap()[:, k0 * IN_C:(k0 + nch) * IN_C])
                    stage = stgp.tile([OUT_C, SG * GCOLS], mybir.dt.bfloat16,
                                      tag="stage")
                    for gl in range(ngr):
                        g = g0 + gl
                        S = stp.tile([128, GCH * W], mybir.dt.bfloat16,
                                     tag="S")
                        iota_bc = (iota_t[:].unsqueeze(1)
                                   .broadcast_to([128, GCH, W]))
                        slot_bc = (slot_t[:, g * GCH:(g + 1) * GCH]
                                   .unsqueeze(2).broadcast_to([128, GCH, W]))
                        nc.vector.scalar_tensor_tensor(
                            S[:], iota_bc, 1.0, slot_bc,
                            mybir.AluOpType.mult, mybir.AluOpType.is_equal)

                        agg = pagg.tile([IN_C, GCOLS], mybir.dt.float32,
                                        tag="agg")
                        for k in range(GCH):
                            blk = k // CPB
                            kk = gl * GCH + k
                            nc.tensor.matmul(
                                agg[:, blk * W:(blk + 1) * W],
                                mt[:, kk * IN_C:(kk + 1) * IN_C],
                                S[:, k * W:(k + 1) * W],
                                start=(k % CPB == 0),
                                stop=(k % CPB == CPB - 1))
                        aggs = sb.tile([IN_C, GCOLS], mybir.dt.bfloat16,
                                       tag="aggs")
                        nc.scalar.activation(aggs[:], agg[:], Copy)

                        hpT = ph.tile([HID_C, GCOLS], mybir.dt.float32,
                                      tag="hp")
                        nc.tensor.matmul(hpT[:], w3_t[:], aggs[:],
                                         start=True, stop=True)
                        hT = sb.tile([HID_C, GCOLS], mybir.dt.bfloat16,
                                     tag="hT")
                        nc.scalar.activation(hT[:], hpT[:], Relu,
                                             bias=b3_t[:])

                        zT = pz.tile([OUT_C, GCOLS], mybir.dt.float32,
                                     tag="zT")
                        nc.tensor.matmul(zT[:], w4_t[:], hT[:],
                                         start=True, stop=True)
                        nc.scalar.activation(
                            stage[:, gl * GCOLS:(gl + 1) * GCOLS],
                            zT[:], Copy)
                    nc.sync.dma_start(
                        zst_d.ap()[:, g0 * GCOLS:g0 * GCOLS + ngr * GCOLS],
                        stage[:, :ngr * GCOLS])

            if loop_reps:
                with tc.For_i(0, loop_reps, 1):
                    body()
            else:
                for _ in range(reps):
                    body()
    nc.compile()
    return nc


def build_layer2(NB, reps=1, loop_reps=0):
    """out^T = agg(z*norm) + b4, staged transposed f32.

    Inputs:  msg [128, NCH*OUT_C] bf16, slot [128, NCH] bf16,
             iota [128, W] bf16, b4c [OUT_C,1] f32
    Output:  ostage [OUT_C, NB*W] f32
    """
    _import_bass()
    NCH = NB * CPB
    NG = NB // GRP
    COLS = NB * W

    nc = bacc.Bacc("TRN2", target_bir_lowering=False, debug=False,
                   num_devices=N_CORES)
    msg_d = nc.dram_tensor("msg", [128, NCH * OUT_C], mybir.dt.bfloat16,
                           kind="ExternalInput")
    slot_d = nc.dram_tensor("slot", [128, NCH], mybir.dt.bfloat16,
                            kind="ExternalInput")
    iota_d = nc.dram_tensor("iota", [128, W], mybir.dt.bfloat16,
                            kind="ExternalInput")
    b4_d = nc.dram_tensor("b4c", [OUT_C, 1], mybir.dt.float32,
                          kind="ExternalInput")
    ost_d = nc.dram_tensor("ostage", [OUT_C, COLS], mybir.dt.float32,
                           kind="ExternalOutput")

    Ident = mybir.ActivationFunctionType.Identity

    with tile.TileContext(nc) as tc:
        with (
            tc.tile_pool(name="const", bufs=1) as constp,
            tc.tile_pool(name="msgs", bufs=3) as msgp,
            tc.tile_pool(name="stg", bufs=2) as stgp,
            tc.tile_pool(name="stmp", bufs=3) as stp,
            tc.tile_pool(name="pagg", bufs=2, space="PSUM") as pagg,
        ):
            iota_t = constp.tile([128, W], mybir.dt.bfloat16)
            nc.sync.dma_start(iota_t[:], iota_d.ap())
            slot_t = constp.tile([128, NCH], mybir.dt.bfloat16)
            nc.sync.dma_start(slot_t[:], slot_d.ap())
            b4_t = constp.tile([OUT_C, 1], mybir.dt.float32)
            nc.sync.dma_start(b4_t[:], b4_d.ap())

            def body():
                for g0 in range(0, NG, SG):
                    ngr = min(SG, NG - g0)
                    k0 = g0 * GCH
                    nch = ngr * GCH
                    mt = msgp.tile([128, SG * GCH * OUT_C], mybir.dt.bfloat16,
                                   tag="mt")
                    nc.sync.dma_start(
                        mt[:, :nch * OUT_C],
                        msg_d.ap()[:, k0 * OUT_C:(k0 + nch) * OUT_C])
                    stage = stgp.tile([OUT_C, SG * GCOLS], mybir.dt.float32,
                                      tag="stage")
                    for gl in range(ngr):
                        g = g0 + gl
                        S = stp.tile([128, GCH * W], mybir.dt.bfloat16,
                                     tag="S")
                        iota_bc = (iota_t[:].unsqueeze(1)
                                   .broadcast_to([128, GCH, W]))
                        slot_bc = (slot_t[:, g * GCH:(g + 1) * GCH]
                                   .unsqueeze(2).broadcast_to([128, GCH, W]))
                        nc.vector.scalar_tensor_tensor(
                            S[:], iota_bc, 1.0, slot_bc,
                            mybir.AluOpType.mult, mybir.AluOpType.is_equal)

                        agg = pagg.tile([OUT_C, GCOLS], mybir.dt.float32,
                                        tag="agg")
                        for k in range(GCH):
                            blk = k // CPB
                            kk = gl * GCH + k
                            nc.tensor.matmul(
                                agg[:, blk * W:(blk + 1) * W],
                                mt[:, kk * OUT_C:(kk + 1) * OUT_C],
                                S[:, k * W:(k + 1) * W],
                                start=(k % CPB == 0),
                                stop=(k % CPB == CPB - 1))
                        nc.scalar.activation(
                            stage[:, gl * GCOLS:(gl + 1) * GCOLS],
                            agg[:], Ident, bias=b4_t[:])
                    nc.sync.dma_start(
                        ost_d.ap()[:, g0 * GCOLS:g0 * GCOLS + ngr * GCOLS],
                        stage[:, :ngr * GCOLS])

            if loop_reps:
                with tc.For_i(0, loop_reps, 1):
                    body()
            else:
                for _ in range(reps):
                    body()
    nc.compile()
    return nc


# ----------------------------------------------------------------------------
# full kernel
# ----------------------------------------------------------------------------

LAST_HW_EXEC_NS = None
LAST_NB = None


def _run(nc, in_maps):
    _import_bass()
    res = bass_utils.run_bass_kernel_spmd(nc, in_maps,
                                          core_ids=list(range(N_CORES)))
    return res.results


def kernel(x, edge_index, W3, b3, W4, b4):
    global LAST_NB
    _import_bass()
    x = np.asarray(x, np.float32)
    prep = preprocess(x, np.asarray(edge_index))
    NB, NCH = prep["NB"], prep["NCH"]
    LAST_NB = NB
    COLS = NB * W

    iota_np = np.tile(np.arange(W, dtype=np.float32), (128, 1)).astype(bf16)
    W3_bf = np.asarray(W3, np.float32).astype(bf16)
    W4_bf = np.asarray(W4, np.float32).astype(bf16)
    b3_c = np.asarray(b3, np.float32).reshape(HID_C, 1)
    b4_c = np.asarray(b4, np.float32).reshape(OUT_C, 1)

    nc1 = build_layer1(NB)
    in1 = []
    for c in prep["cores"]:
        in1.append(dict(
            msg=_stage_msgs(x, c["src_t"], c["e_norm"]),
            slot=c["meta_slot"], iota=iota_np,
            w3=W3_bf, b3c=b3_c, w4=W4_bf))
    res1 = _run(nc1, in1)
    # z, transposed per core: [OUT_C, COLS] -> all cores' columns concatenated
    zT = np.concatenate([np.asarray(r["zstage"]) for r in res1], axis=1)
    z = np.ascontiguousarray(zT.T)            # [8*COLS, OUT_C] bf16

    nc2 = build_layer2(NB)
    in2 = []
    for c in prep["cores"]:
        in2.append(dict(
            msg=_stage_msgs(z, c["g2_ind"], c["e_norm"]),
            slot=c["meta_slot"], iota=iota_np, b4c=b4_c))
    res2 = _run(nc2, in2)
    oT = np.concatenate([np.asarray(r["ostage"]) for r in res2], axis=1)

    sr = prep["stage_row"]
    out = np.ascontiguousarray(oT.T)[sr]
    return out.astype(np.float32)


# revision 8
# speedup vs baseline: 1.2761x; 1.2761x over previous
"""Trainium2 Bass kernel for a 2-layer GCN decoder (nn_GCNDecoder).

Strategy (8 NeuronCores, SPMD):
  - Destination nodes sharded 8 ways. Edges (with self-loops) partitioned by
    dst shard, grouped by dst into blocks of <=32 distinct dsts (W) x <=512
    edge lanes (CPB=4 chunks of 128). 8 blocks form a "group" = 256 PSUM
    columns processed by one weight-chain pass.
  - GCN normalization norm_e = dinv[src]*dinv[dst] is folded into the staged
    messages on the host (msg = x[src]*norm). The per-chunk selection matrix
    S[lane, slot] = (iota==slot_id) is built on the DVE with ONE broadcast
    scalar_tensor_tensor op per group; segment-sum aggregation is a PE matmul
    agg[Cin, slot] += M^T S with PSUM accumulation.
  - Layer 1 fuses BOTH weight matmuls transposed (constant stationary):
    z^T = W4^T relu(W3^T agg + b3), so layer-2 messages are 64-wide.
    Layer 2 is aggregation + b4 only.
  - Source features are staged per-edge-lane by the host (halo exchange):
    layer 1 messages from x*norm, layer 2 messages from z*norm, where z is
    re-staged between the two device programs.

The host does: integer packing/sorting, degree->norm prep, staging of
per-lane message rows, and output unpermutation.
"""

import os
import sys
import numpy as np
import ml_dtypes

bf16 = ml_dtypes.bfloat16
f8 = ml_dtypes.float8_e4m3

# message dtype: fp8 e4m3 with a power-of-two prescale (descaled on-chip in
# the PSUM->SBUF copy). Halves message DMA vs bf16; rel err ~1.5e-2.
MSG_SCALE = 16.0

# problem constants (spec: nn_GCNDecoder_32959579030036)
N_NODES = 100000
IN_C = 64
HID_C = 128
OUT_C = 64
N_CORES = 8
SHARD = N_NODES // N_CORES   # 12500

W = 32                        # dst slots per block
CPB = 4                       # chunks (of 128 edge lanes) per block
SLOTS = CPB * 128             # 512 edge lanes per block
GRP = 8                       # blocks per group (8*W = 256 psum columns)
GCOLS = GRP * W               # 256
GCH = GRP * CPB               # 32 chunks per group
SG = 8                        # groups per output stage chunk / msg DMA batch

_BASS_READY = False


def _import_bass():
    global _BASS_READY, bacc, tile, mybir, bass_utils
    if _BASS_READY:
        return
    for p in ("/opt/trn_rl_repo", "/opt/pypackages"):
        if os.path.isdir(p) and p not in sys.path:
            sys.path.append(p)
    import concourse.bacc as bacc
    import concourse.tile as tile
    import concourse.mybir as mybir
    from concourse import bass_utils
    _BASS_READY = True


# ----------------------------------------------------------------------------
# host-side packing
# ----------------------------------------------------------------------------

def _pack_core(src, dst, norm):
    order = np.argsort(dst, kind="stable")
    src, dst, norm = src[order], dst[order], norm[order]
    uniq, seg_start = np.unique(dst, return_index=True)
    seg_end = np.append(seg_start[1:], len(dst))
    seg_len = seg_end - seg_start
    assert seg_len.max() <= SLOTS, "node in-degree exceeds block capacity"

    blocks, cur, cur_slots = [], [], 0
    for i in range(len(uniq)):
        if cur and (cur_slots + seg_len[i] > SLOTS or len(cur) >= W):
            blocks.append(cur)
            cur, cur_slots = [], 0
        cur.append(i)
        cur_slots += seg_len[i]
    if cur:
        blocks.append(cur)

    nb = len(blocks)
    e_src = np.zeros((nb, SLOTS), np.int64)
    e_slot = np.zeros((nb, SLOTS), np.float32)
    e_norm = np.zeros((nb, SLOTS), np.float32)
    slot_node = np.full((nb, W), -1, np.int64)
    for b, segs in enumerate(blocks):
        ps, pl, pn = [], [], []
        for s_local, i in enumerate(segs):
            sl = slice(seg_start[i], seg_end[i])
            ps.append(src[sl])
            pl.append(np.full(seg_len[i], s_local, np.float32))
            pn.append(norm[sl])
            slot_node[b, s_local] = uniq[i]
        bs, bslot, bn = map(np.concatenate, (ps, pl, pn))
        o = np.argsort(bs, kind="stable")
        n = len(bs)
        e_src[b, :n] = bs[o]
        e_slot[b, :n] = bslot[o]
        e_norm[b, :n] = bn[o]
    return dict(nb=nb, e_src=e_src, e_slot=e_slot, e_norm=e_norm,
                slot_node=slot_node)


def preprocess(x, edge_index):
    src = np.asarray(edge_index[0], np.int64)
    dst = np.asarray(edge_index[1], np.int64)
    loops = np.arange(N_NODES, dtype=np.int64)
    src_all = np.concatenate([src, loops])
    dst_all = np.concatenate([dst, loops])
    deg = np.bincount(dst_all, minlength=N_NODES).astype(np.float32)
    dinv = 1.0 / np.sqrt(deg)
    norm_all = (dinv[src_all] * dinv[dst_all]).astype(np.float32)

    shard_of = dst_all // SHARD
    cores = []
    for c in range(N_CORES):
        m = shard_of == c
        cores.append(_pack_core(src_all[m], dst_all[m], norm_all[m]))

    NB = max(c["nb"] for c in cores)
    NB = ((NB + GRP - 1) // GRP) * GRP       # pad to whole groups

    for c in cores:
        pad = NB - c["nb"]
        if pad:
            c["e_src"] = np.concatenate([c["e_src"], np.zeros((pad, SLOTS), np.int64)])
            c["e_slot"] = np.concatenate([c["e_slot"], np.zeros((pad, SLOTS), np.float32)])
            c["e_norm"] = np.concatenate([c["e_norm"], np.zeros((pad, SLOTS), np.float32)])
            c["slot_node"] = np.concatenate([c["slot_node"], np.full((pad, W), -1, np.int64)])

    # stage_row[node] = flat column index of that node's slot in the
    # concatenated [cores x NB*W] transposed stage
    stage_row = np.full(N_NODES, -1, np.int64)
    for ci, c in enumerate(cores):
        sn = c["slot_node"].ravel()
        valid = sn >= 0
        stage_row[sn[valid]] = ci * NB * W + np.nonzero(valid)[0]
    assert (stage_row >= 0).all()

    NCH = NB * CPB
    out = dict(NB=NB, NCH=NCH, stage_row=stage_row, cores=[])
    for c in cores:
        e_src = c["e_src"].reshape(NCH, 128)
        meta_slot = np.ascontiguousarray(
            c["e_slot"].reshape(NCH, 128).T).astype(bf16)   # [128,NCH]
        e_norm = np.ascontiguousarray(c["e_norm"].reshape(NCH, 128).T)
        g2 = stage_row[e_src]                                # [NCH,128]
        g2_ind = np.ascontiguousarray(g2.T)                  # [128,NCH]
        src_t = np.ascontiguousarray(e_src.T)                # [128,NCH]
        out["cores"].append(dict(meta_slot=meta_slot, e_norm=e_norm,
                                 g2_ind=g2_ind, src_t=src_t))
    return out


def _stage_msgs(rows, ind, norm):
    """rows [N, C] f32/bf16; ind [128, NCH]; norm [128, NCH] f32
    -> [128, NCH*C] fp8 messages with norm (and MSG_SCALE) folded in."""
    C = rows.shape[1]
    m = rows[ind].astype(np.float32)          # [128, NCH, C]
    m *= (MSG_SCALE * norm)[:, :, None]
    return np.ascontiguousarray(m.astype(f8).reshape(128, -1))


# ----------------------------------------------------------------------------
# device programs
# ----------------------------------------------------------------------------

def build_layer1(NB, reps=1, loop_reps=0):
    """agg(x*norm) -> z^T = W4^T relu(W3^T agg + b3); z staged transposed.

    Inputs:  msg [128, NCH*IN_C] bf16, slot [128, NCH] bf16,
             iota [128, W] bf16, w3 [IN_C, HID_C] bf16, b3c [HID_C,1] f32,
             w4 [HID_C, OUT_C] bf16
    Output:  zstage [OUT_C, NB*W] bf16   (transposed: feature-major)
    """
    _import_bass()
    NCH = NB * CPB
    NG = NB // GRP
    COLS = NB * W

    nc = bacc.Bacc("TRN2", target_bir_lowering=False, debug=False,
                   num_devices=N_CORES)
    msg_d = nc.dram_tensor("msg", [128, NCH * IN_C], mybir.dt.float8e4,
                           kind="ExternalInput")
    slot_d = nc.dram_tensor("slot", [128, NCH], mybir.dt.bfloat16,
                            kind="ExternalInput")
    iota_d = nc.dram_tensor("iota", [128, W], mybir.dt.bfloat16,
                            kind="ExternalInput")
    w3_d = nc.dram_tensor("w3", [IN_C, HID_C], mybir.dt.bfloat16,
                          kind="ExternalInput")
    b3_d = nc.dram_tensor("b3c", [HID_C, 1], mybir.dt.float32,
                          kind="ExternalInput")
    w4_d = nc.dram_tensor("w4", [HID_C, OUT_C], mybir.dt.bfloat16,
                          kind="ExternalInput")
    zst_d = nc.dram_tensor("zstage", [OUT_C, COLS], mybir.dt.bfloat16,
                           kind="ExternalOutput")

    Relu = mybir.ActivationFunctionType.Relu
    Copy = mybir.ActivationFunctionType.Copy

    with tile.TileContext(nc) as tc:
        with (
            tc.tile_pool(name="const", bufs=1) as constp,
            tc.tile_pool(name="msgs", bufs=3) as msgp,
            tc.tile_pool(name="stg", bufs=2) as stgp,
            tc.tile_pool(name="sbuf", bufs=3) as sb,
            tc.tile_pool(name="stmp", bufs=3) as stp,
            tc.tile_pool(name="pagg", bufs=2, space="PSUM") as pagg,
            tc.tile_pool(name="ph", bufs=2, space="PSUM") as ph,
            tc.tile_pool(name="pz", bufs=2, space="PSUM") as pz,
        ):
            iota_t = constp.tile([128, W], mybir.dt.bfloat16)
            nc.sync.dma_start(iota_t[:], iota_d.ap())
            slot_t = constp.tile([128, NCH], mybir.dt.bfloat16)
            nc.sync.dma_start(slot_t[:], slot_d.ap())
            w3_t = constp.tile([IN_C, HID_C], mybir.dt.bfloat16)
            nc.sync.dma_start(w3_t[:], w3_d.ap())
            b3_t = constp.tile([HID_C, 1], mybir.dt.float32)
            nc.sync.dma_start(b3_t[:], b3_d.ap())
            w4_t = constp.tile([HID_C, OUT_C], mybir.dt.bfloat16)
            nc.sync.dma_start(w4_t[:], w4_d.ap())

            def body():
                for g0 in range(0, NG, SG):
                    ngr = min(SG, NG - g0)
                    # message DMA for SG groups at once
                    k0 = g0 * GCH
                    nch = ngr * GCH
                    mt = msgp.tile([128, SG * GCH * IN_C], mybir.dt.float8e4,
                                   tag="mt")
                    nc.sync.dma_start(
                        mt[:, :nch * IN_C],
                        msg_d.ap()[:, k0 * IN_C:(k0 + nch) * IN_C])
                    stage = stgp.tile([OUT_C, SG * GCOLS], mybir.dt.bfloat16,
                                      tag="stage")
                    for gl in range(ngr):
                        g = g0 + gl
                        S = stp.tile([128, GCH * W], mybir.dt.float8e4,
                                     tag="S")
                        iota_bc = (iota_t[:].unsqueeze(1)
                                   .broadcast_to([128, GCH, W]))
                        slot_bc = (slot_t[:, g * GCH:(g + 1) * GCH]
                                   .unsqueeze(2).broadcast_to([128, GCH, W]))
                        nc.vector.scalar_tensor_tensor(
                            S[:], iota_bc, 1.0, slot_bc,
                            mybir.AluOpType.mult, mybir.AluOpType.is_equal)

                        agg = pagg.tile([IN_C, GCOLS], mybir.dt.float32,
                                        tag="agg")
                        for k in range(GCH):
                            blk = k // CPB
                            kk = gl * GCH + k
                            nc.tensor.matmul(
                                agg[:, blk * W:(blk + 1) * W],
                                mt[:, kk * IN_C:(kk + 1) * IN_C],
                                S[:, k * W:(k + 1) * W],
                                start=(k % CPB == 0),
                                stop=(k % CPB == CPB - 1))
                        aggs = sb.tile([IN_C, GCOLS], mybir.dt.bfloat16,
                                       tag="aggs")
                        nc.scalar.activation(aggs[:], agg[:], Copy,
                                             scale=1.0 / MSG_SCALE)

                        hpT = ph.tile([HID_C, GCOLS], mybir.dt.float32,
                                      tag="hp")
                        nc.tensor.matmul(hpT[:], w3_t[:], aggs[:],
                                         start=True, stop=True)
                        hT = sb.tile([HID_C, GCOLS], mybir.dt.bfloat16,
                                     tag="hT")
                        nc.scalar.activation(hT[:], hpT[:], Relu,
                                             bias=b3_t[:])

                        zT = pz.tile([OUT_C, GCOLS], mybir.dt.float32,
                                     tag="zT")
                        nc.tensor.matmul(zT[:], w4_t[:], hT[:],
                                         start=True, stop=True)
                        nc.scalar.activation(
                            stage[:, gl * GCOLS:(gl + 1) * GCOLS],
                            zT[:], Copy)
                    nc.sync.dma_start(
                        zst_d.ap()[:, g0 * GCOLS:g0 * GCOLS + ngr * GCOLS],
                        stage[:, :ngr * GCOLS])

            if loop_reps:
                with tc.For_i(0, loop_reps, 1):
                    body()
            else:
                for _ in range(reps):
                    body()
    nc.compile()
    return nc


def build_layer2(NB, reps=1, loop_reps=0):
    """out^T = agg(z*norm) + b4, staged transposed f32.

    Inputs:  msg [128, NCH*OUT_C] bf16, slot [128, NCH] bf16,
             iota [128, W] bf16, b4c [OUT_C,1] f32
    Output:  ostage [OUT_C, NB*W] f32
    """
    _import_bass()
    NCH = NB * CPB
    NG = NB // GRP
    COLS = NB * W

    nc = bacc.Bacc("TRN2", target_bir_lowering=False, debug=False,
                   num_devices=N_CORES)
    msg_d = nc.dram_tensor("msg", [128, NCH * OUT_C], mybir.dt.float8e4,
                           kind="ExternalInput")
    slot_d = nc.dram_tensor("slot", [128, NCH], mybir.dt.bfloat16,
                            kind="ExternalInput")
    iota_d = nc.dram_tensor("iota", [128, W], mybir.dt.bfloat16,
                            kind="ExternalInput")
    b4_d = nc.dram_tensor("b4c", [OUT_C, 1], mybir.dt.float32,
                          kind="ExternalInput")
    ost_d = nc.dram_tensor("ostage", [OUT_C, COLS], mybir.dt.float32,
                           kind="ExternalOutput")

    Ident = mybir.ActivationFunctionType.Identity

    with tile.TileContext(nc) as tc:
        with (
            tc.tile_pool(name="const", bufs=1) as constp,
            tc.tile_pool(name="msgs", bufs=3) as msgp,
            tc.tile_pool(name="stg", bufs=2) as stgp,
            tc.tile_pool(name="stmp", bufs=3) as stp,
            tc.tile_pool(name="pagg", bufs=2, space="PSUM") as pagg,
        ):
            iota_t = constp.tile([128, W], mybir.dt.bfloat16)
            nc.sync.dma_start(iota_t[:], iota_d.ap())
            slot_t = constp.tile([128, NCH], mybir.dt.bfloat16)
            nc.sync.dma_start(slot_t[:], slot_d.ap())
            b4_t = constp.tile([OUT_C, 1], mybir.dt.float32)
            nc.sync.dma_start(b4_t[:], b4_d.ap())

            def body():
                for g0 in range(0, NG, SG):
                    ngr = min(SG, NG - g0)
                    k0 = g0 * GCH
                    nch = ngr * GCH
                    mt = msgp.tile([128, SG * GCH * OUT_C], mybir.dt.float8e4,
                                   tag="mt")
                    nc.sync.dma_start(
                        mt[:, :nch * OUT_C],
                        msg_d.ap()[:, k0 * OUT_C:(k0 + nch) * OUT_C])
                    stage = stgp.tile([OUT_C, SG * GCOLS], mybir.dt.float32,
                                      tag="stage")
                    for gl in range(ngr):
                        g = g0 + gl
                        S = stp.tile([128, GCH * W], mybir.dt.float8e4,
                                     tag="S")
                        iota_bc = (iota_t[:].unsqueeze(1)
                                   .broadcast_to([128, GCH, W]))
                        slot_bc = (slot_t[:, g * GCH:(g + 1) * GCH]
                                   .unsqueeze(2).broadcast_to([128, GCH, W]))
                        nc.vector.scalar_tensor_tensor(
                            S[:], iota_bc, 1.0, slot_bc,
                            mybir.AluOpType.mult, mybir.AluOpType.is_equal)

                        agg = pagg.tile([OUT_C, GCOLS], mybir.dt.float32,
                                        tag="agg")
                        for k in range(GCH):
                            blk = k // CPB
                            kk = gl * GCH + k
                            nc.tensor.matmul(
                                agg[:, blk * W:(blk + 1) * W],
                                mt[:, kk * OUT_C:(kk + 1) * OUT_C],
                                S[:, k * W:(k + 1) * W],
                                start=(k % CPB == 0),
                                stop=(k % CPB == CPB - 1))
                        nc.scalar.activation(
                            stage[:, gl * GCOLS:(gl + 1) * GCOLS],
                            agg[:], Ident, bias=b4_t[:],
                            scale=1.0 / MSG_SCALE)
                    nc.sync.dma_start(
                        ost_d.ap()[:, g0 * GCOLS:g0 * GCOLS + ngr * GCOLS],
                        stage[:, :ngr * GCOLS])

            if loop_reps:
                with tc.For_i(0, loop_reps, 1):
                    body()
            else:
                for _ in range(reps):
                    body()
    nc.compile()
    return nc


# ----------------------------------------------------------------------------
# full kernel
# ----------------------------------------------------------------------------

LAST_HW_EXEC_NS = None
LAST_NB = None


def _run(nc, in_maps):
    _import_bass()
    res = bass_utils.run_bass_kernel_spmd(nc, in_maps,
                                          core_ids=list(range(N_CORES)))
    return res.results


def kernel(x, edge_index, W3, b3, W4, b4):
    global LAST_NB
    _import_bass()
    x = np.asarray(x, np.float32)
    prep = preprocess(x, np.asarray(edge_index))
    NB, NCH = prep["NB"], prep["NCH"]
    LAST_NB = NB
    COLS = NB * W

    iota_np = np.tile(np.arange(W, dtype=np.float32), (128, 1)).astype(bf16)
    W3_bf = np.asarray(W3, np.float32).astype(bf16)
    W4_bf = np.asarray(W4, np.float32).astype(bf16)
    b3_c = np.asarray(b3, np.float32).reshape(HID_C, 1)
    b4_c = np.asarray(b4, np.float32).reshape(OUT_C, 1)

    nc1 = build_layer1(NB)
    in1 = []
    for c in prep["cores"]:
        in1.append(dict(
            msg=_stage_msgs(x, c["src_t"], c["e_norm"]),
            slot=c["meta_slot"], iota=iota_np,
            w3=W3_bf, b3c=b3_c, w4=W4_bf))
    res1 = _run(nc1, in1)
    # z, transposed per core: [OUT_C, COLS] -> all cores' columns concatenated
    zT = np.concatenate([np.asarray(r["zstage"]) for r in res1], axis=1)
    z = np.ascontiguousarray(zT.T)            # [8*COLS, OUT_C] bf16

    nc2 = build_layer2(NB)
    in2 = []
    for c in prep["cores"]:
        in2.append(dict(
            msg=_stage_msgs(z, c["g2_ind"], c["e_norm"]),
            slot=c["meta_slot"], iota=iota_np, b4c=b4_c))
    res2 = _run(nc2, in2)
    oT = np.concatenate([np.asarray(r["ostage"]) for r in res2], axis=1)

    sr = prep["stage_row"]
    out = np.ascontiguousarray(oT.T)[sr]
    return out.astype(np.float32)


# revision 20
# speedup vs baseline: 1.6695x; 1.3082x over previous
"""Trainium2 Bass kernel for a 2-layer GCN decoder (nn_GCNDecoder).

Strategy (8 NeuronCores, SPMD):
  - Destination nodes sharded 8 ways. Edges (with self-loops) partitioned by
    dst shard, grouped by dst into blocks of <=32 distinct dsts (W) x <=512
    edge lanes (CPB=4 chunks of 128). 8 blocks form a "group" = 256 PSUM
    columns processed by one weight-chain pass.
  - GCN normalization norm_e = dinv[src]*dinv[dst] is folded into the staged
    messages on the host (msg = x[src]*norm). The per-chunk selection matrix
    S[lane, slot] = (iota==slot_id) is built on the DVE with ONE broadcast
    scalar_tensor_tensor op per group; segment-sum aggregation is a PE matmul
    agg[Cin, slot] += M^T S with PSUM accumulation.
  - Layer 1 fuses BOTH weight matmuls transposed (constant stationary):
    z^T = W4^T relu(W3^T agg + b3), so layer-2 messages are 64-wide.
    Layer 2 is aggregation + b4 only.
  - Source features are staged per-edge-lane by the host (halo exchange):
    layer 1 messages from x*norm, layer 2 messages from z*norm, where z is
    re-staged between the two device programs.

The host does: integer packing/sorting, degree->norm prep, staging of
per-lane message rows, and output unpermutation.
"""

import os
import sys
import numpy as np
import ml_dtypes

bf16 = ml_dtypes.bfloat16
f8 = ml_dtypes.float8_e4m3

# message dtype: fp8 e4m3 with a power-of-two prescale (descaled on-chip in
# the PSUM->SBUF copy). Halves message DMA vs bf16; rel err ~1.5e-2.
MSG_SCALE = 16.0

# problem constants (spec: nn_GCNDecoder_32959579030036)
N_NODES = 100000
IN_C = 64
HID_C = 128
OUT_C = 64
N_CORES = 8
SHARD = N_NODES // N_CORES   # 12500

W = 16                        # dst slots per block
CPB = 2                       # chunks (of 128 edge lanes) per block
SLOTS = CPB * 128             # 256 edge lanes per block
GRP = 32                      # blocks per group (32*W = 512 psum columns)
GCOLS = GRP * W               # 256
GCH = GRP * CPB               # 32 chunks per group
SG = 2                        # groups per output stage chunk / msg DMA batch

_BASS_READY = False


def _import_bass():
    global _BASS_READY, bacc, tile, mybir, bass_utils
    if _BASS_READY:
        return
    for p in ("/opt/trn_rl_repo", "/opt/pypackages"):
        if os.path.isdir(p) and p not in sys.path:
            sys.path.append(p)
    import concourse.bacc as bacc
    import concourse.tile as tile
    import concourse.mybir as mybir
    from concourse import bass_utils
    _BASS_READY = True


# ----------------------------------------------------------------------------
# host-side packing
# ----------------------------------------------------------------------------

def _pack_core(src, dst, norm):
    order = np.argsort(dst, kind="stable")
    src, dst, norm = src[order], dst[order], norm[order]
    uniq, seg_start = np.unique(dst, return_index=True)
    seg_end = np.append(seg_start[1:], len(dst))
    seg_len = seg_end - seg_start
    assert seg_len.max() <= SLOTS, "node in-degree exceeds block capacity"

    blocks, cur, cur_slots = [], [], 0
    for i in range(len(uniq)):
        if cur and (cur_slots + seg_len[i] > SLOTS or len(cur) >= W):
            blocks.append(cur)
            cur, cur_slots = [], 0
        cur.append(i)
        cur_slots += seg_len[i]
    if cur:
        blocks.append(cur)

    nb = len(blocks)
    e_src = np.zeros((nb, SLOTS), np.int64)
    e_slot = np.zeros((nb, SLOTS), np.float32)
    e_norm = np.zeros((nb, SLOTS), np.float32)
    slot_node = np.full((nb, W), -1, np.int64)
    for b, segs in enumerate(blocks):
        ps, pl, pn = [], [], []
        for s_local, i in enumerate(segs):
            sl = slice(seg_start[i], seg_end[i])
            ps.append(src[sl])
            pl.append(np.full(seg_len[i], s_local, np.float32))
            pn.append(norm[sl])
            slot_node[b, s_local] = uniq[i]
        bs, bslot, bn = map(np.concatenate, (ps, pl, pn))
        o = np.argsort(bs, kind="stable")
        n = len(bs)
        e_src[b, :n] = bs[o]
        e_slot[b, :n] = bslot[o]
        e_norm[b, :n] = bn[o]
    return dict(nb=nb, e_src=e_src, e_slot=e_slot, e_norm=e_norm,
                slot_node=slot_node)


def preprocess(x, edge_index):
    src = np.asarray(edge_index[0], np.int64)
    dst = np.asarray(edge_index[1], np.int64)
    loops = np.arange(N_NODES, dtype=np.int64)
    src_all = np.concatenate([src, loops])
    dst_all = np.concatenate([dst, loops])
    deg = np.bincount(dst_all, minlength=N_NODES).astype(np.float32)
    dinv = 1.0 / np.sqrt(deg)
    norm_all = (dinv[src_all] * dinv[dst_all]).astype(np.float32)

    shard_of = dst_all // SHARD
    cores = []
    for c in range(N_CORES):
        m = shard_of == c
        cores.append(_pack_core(src_all[m], dst_all[m], norm_all[m]))

    NB = max(c["nb"] for c in cores)
    NB = ((NB + GRP - 1) // GRP) * GRP       # pad to whole groups

    for c in cores:
        pad = NB - c["nb"]
        if pad:
            c["e_src"] = np.concatenate([c["e_src"], np.zeros((pad, SLOTS), np.int64)])
            c["e_slot"] = np.concatenate([c["e_slot"], np.zeros((pad, SLOTS), np.float32)])
            c["e_norm"] = np.concatenate([c["e_norm"], np.zeros((pad, SLOTS), np.float32)])
            c["slot_node"] = np.concatenate([c["slot_node"], np.full((pad, W), -1, np.int64)])

    # stage_row[node] = flat column index of that node's slot in the
    # concatenated [cores x NB*W] transposed stage
    stage_row = np.full(N_NODES, -1, np.int64)
    for ci, c in enumerate(cores):
        sn = c["slot_node"].ravel()
        valid = sn >= 0
        stage_row[sn[valid]] = ci * NB * W + np.nonzero(valid)[0]
    assert (stage_row >= 0).all()

    NCH = NB * CPB
    out = dict(NB=NB, NCH=NCH, stage_row=stage_row, cores=[])
    for c in cores:
        e_src = c["e_src"].reshape(NCH, 128)
        meta_slot = np.ascontiguousarray(
            c["e_slot"].reshape(NCH, 128).T).astype(bf16)   # [128,NCH]
        e_norm = np.ascontiguousarray(c["e_norm"].reshape(NCH, 128).T)
        g2 = stage_row[e_src]                                # [NCH,128]
        g2_ind = np.ascontiguousarray(g2.T)                  # [128,NCH]
        src_t = np.ascontiguousarray(e_src.T)                # [128,NCH]
        out["cores"].append(dict(meta_slot=meta_slot, e_norm=e_norm,
                                 g2_ind=g2_ind, src_t=src_t))
    return out


def _stage_msgs(rows, ind, norm):
    """rows [N, C] f32/bf16; ind [128, NCH]; norm [128, NCH] f32
    -> [128, NCH*C] fp8 messages with norm (and MSG_SCALE) folded in."""
    C = rows.shape[1]
    m = rows[ind].astype(np.float32)          # [128, NCH, C]
    m *= (MSG_SCALE * norm)[:, :, None]
    return np.ascontiguousarray(m.astype(f8).reshape(128, -1))


# ----------------------------------------------------------------------------
# device programs
# ----------------------------------------------------------------------------

def build_layer1(NB, reps=1, loop_reps=0):
    """agg(x*norm) -> z^T = W4^T relu(W3^T agg + b3); z staged transposed.

    Inputs:  msg [128, NCH*IN_C] bf16, slot [128, NCH] bf16,
             iota [128, W] bf16, w3 [IN_C, HID_C] bf16, b3c [HID_C,1] f32,
             w4 [HID_C, OUT_C] bf16
    Output:  zstage [OUT_C, NB*W] bf16   (transposed: feature-major)
    """
    _import_bass()
    NCH = NB * CPB
    NG = NB // GRP
    COLS = NB * W

    nc = bacc.Bacc("TRN2", target_bir_lowering=False, debug=False,
                   num_devices=N_CORES)
    msg_d = nc.dram_tensor("msg", [128, NCH * IN_C], mybir.dt.float8e4,
                           kind="ExternalInput")
    slot_d = nc.dram_tensor("slot", [128, NCH], mybir.dt.bfloat16,
                            kind="ExternalInput")
    iota_d = nc.dram_tensor("iota", [128, W], mybir.dt.bfloat16,
                            kind="ExternalInput")
    w3_d = nc.dram_tensor("w3", [IN_C, HID_C], mybir.dt.bfloat16,
                          kind="ExternalInput")
    b3_d = nc.dram_tensor("b3c", [HID_C, 1], mybir.dt.float32,
                          kind="ExternalInput")
    w4_d = nc.dram_tensor("w4", [HID_C, OUT_C], mybir.dt.bfloat16,
                          kind="ExternalInput")
    zst_d = nc.dram_tensor("zstage", [OUT_C, COLS], mybir.dt.bfloat16,
                           kind="ExternalOutput")

    Relu = mybir.ActivationFunctionType.Relu
    Copy = mybir.ActivationFunctionType.Copy

    with tile.TileContext(nc) as tc:
        with (
            tc.tile_pool(name="const", bufs=1) as constp,
            tc.tile_pool(name="msgs", bufs=4) as msgp,
            tc.tile_pool(name="stg", bufs=2) as stgp,
            tc.tile_pool(name="sbuf", bufs=3) as sb,
            tc.tile_pool(name="stmp", bufs=8) as stp,
            tc.tile_pool(name="pagg", bufs=3, space="PSUM") as pagg,
            tc.tile_pool(name="ph", bufs=2, space="PSUM") as ph,
            tc.tile_pool(name="pz", bufs=2, space="PSUM") as pz,
        ):
            iota_t = constp.tile([128, W], mybir.dt.bfloat16)
            nc.scalar.dma_start(iota_t[:], iota_d.ap())
            slot_t = constp.tile([128, NCH], mybir.dt.bfloat16)
            nc.scalar.dma_start(slot_t[:], slot_d.ap())
            w3_t = constp.tile([IN_C, HID_C], mybir.dt.bfloat16)
            nc.scalar.dma_start(w3_t[:], w3_d.ap())
            b3_t = constp.tile([HID_C, 1], mybir.dt.float32)
            nc.scalar.dma_start(b3_t[:], b3_d.ap())
            w4_t = constp.tile([HID_C, OUT_C], mybir.dt.bfloat16)
            nc.scalar.dma_start(w4_t[:], w4_d.ap())

            def body():
                for g0 in range(0, NG, SG):
                    ngr = min(SG, NG - g0)
                    # message DMA for SG groups at once
                    k0 = g0 * GCH
                    nch = ngr * GCH
                    mt = msgp.tile([128, SG * GCH * IN_C], mybir.dt.float8e4,
                                   tag="mt")
                    nc.sync.dma_start(
                        mt[:, :nch * IN_C],
                        msg_d.ap()[:, k0 * IN_C:(k0 + nch) * IN_C])
                    stage = stgp.tile([OUT_C, SG * GCOLS], mybir.dt.bfloat16,
                                      tag="stage")
                    for gl in range(ngr):
                        g = g0 + gl
                        S = stp.tile([128, GCH * W], mybir.dt.float8e4,
                                     tag="S")
                        iota_bc = (iota_t[:].unsqueeze(1)
                                   .broadcast_to([128, GCH, W]))
                        slot_bc = (slot_t[:, g * GCH:(g + 1) * GCH]
                                   .unsqueeze(2).broadcast_to([128, GCH, W]))
                        nc.vector.scalar_tensor_tensor(
                            S[:], iota_bc, 1.0, slot_bc,
                            mybir.AluOpType.mult, mybir.AluOpType.is_equal)

                        agg = pagg.tile([IN_C, GCOLS], mybir.dt.float32,
                                        tag="agg")
                        for b in range(GRP):
                            kk = gl * GCH + 2 * b
                            lhs = (mt[:, kk * IN_C:(kk + 2) * IN_C]
                                   .rearrange("p (two f) -> p two f", two=2))
                            rhs = (S[:, 2 * b * W:(2 * b + 2) * W]
                                   .rearrange("p (two f) -> p two f", two=2))
                            nc.tensor.matmul(
                                agg[:, b * W:(b + 1) * W], lhs, rhs,
                                start=True, stop=True,
                                perf_mode=mybir.MatmulPerfMode.DoubleRow)
                        aggs = sb.tile([IN_C, GCOLS], mybir.dt.bfloat16,
                                       tag="aggs")
                        nc.vector.tensor_scalar_mul(aggs[:], agg[:],
                                                    1.0 / MSG_SCALE)

                        hpT = ph.tile([HID_C, GCOLS], mybir.dt.float32,
                                      tag="hp")
                        nc.tensor.matmul(hpT[:], w3_t[:], aggs[:],
                                         start=True, stop=True)
                        hT = sb.tile([HID_C, GCOLS], mybir.dt.bfloat16,
                                     tag="hT")
                        nc.scalar.activation(hT[:], hpT[:], Relu,
                                             bias=b3_t[:])

                        zT = pz.tile([OUT_C, GCOLS], mybir.dt.float32,
                                     tag="zT")
                        nc.tensor.matmul(zT[:], w4_t[:], hT[:],
                                         start=True, stop=True)
                        nc.scalar.activation(
                            stage[:, gl * GCOLS:(gl + 1) * GCOLS],
                            zT[:], Copy)
                    nc.scalar.dma_start(
                        zst_d.ap()[:, g0 * GCOLS:g0 * GCOLS + ngr * GCOLS],
                        stage[:, :ngr * GCOLS])

            if loop_reps:
                with tc.For_i(0, loop_reps, 1):
                    body()
            else:
                for _ in range(reps):
                    body()
    nc.compile()
    return nc


def build_layer2(NB, reps=1, loop_reps=0):
    """out^T = agg(z*norm) + b4, staged transposed f32.

    Inputs:  msg [128, NCH*OUT_C] bf16, slot [128, NCH] bf16,
             iota [128, W] bf16, b4c [OUT_C,1] f32
    Output:  ostage [OUT_C, NB*W] f32
    """
    _import_bass()
    NCH = NB * CPB
    NG = NB // GRP
    COLS = NB * W

    nc = bacc.Bacc("TRN2", target_bir_lowering=False, debug=False,
                   num_devices=N_CORES)
    msg_d = nc.dram_tensor("msg", [128, NCH * OUT_C], mybir.dt.float8e4,
                           kind="ExternalInput")
    slot_d = nc.dram_tensor("slot", [128, NCH], mybir.dt.bfloat16,
                            kind="ExternalInput")
    iota_d = nc.dram_tensor("iota", [128, W], mybir.dt.bfloat16,
                            kind="ExternalInput")
    b4_d = nc.dram_tensor("b4c", [OUT_C, 1], mybir.dt.float32,
                          kind="ExternalInput")
    ost_d = nc.dram_tensor("ostage", [OUT_C, COLS], mybir.dt.bfloat16,
                           kind="ExternalOutput")

    Ident = mybir.ActivationFunctionType.Identity

    with tile.TileContext(nc) as tc:
        with (
            tc.tile_pool(name="const", bufs=1) as constp,
            tc.tile_pool(name="msgs", bufs=4) as msgp,
            tc.tile_pool(name="stg", bufs=2) as stgp,
            tc.tile_pool(name="stmp", bufs=8) as stp,
            tc.tile_pool(name="pagg", bufs=3, space="PSUM") as pagg,
        ):
            iota_t = constp.tile([128, W], mybir.dt.bfloat16)
            nc.scalar.dma_start(iota_t[:], iota_d.ap())
            slot_t = constp.tile([128, NCH], mybir.dt.bfloat16)
            nc.scalar.dma_start(slot_t[:], slot_d.ap())
            b4_t = constp.tile([OUT_C, 1], mybir.dt.float32)
            nc.scalar.dma_start(b4_t[:], b4_d.ap())

            def body():
                for g0 in range(0, NG, SG):
                    ngr = min(SG, NG - g0)
                    k0 = g0 * GCH
                    nch = ngr * GCH
                    mt = msgp.tile([128, SG * GCH * OUT_C], mybir.dt.float8e4,
                                   tag="mt")
                    nc.sync.dma_start(
                        mt[:, :nch * OUT_C],
                        msg_d.ap()[:, k0 * OUT_C:(k0 + nch) * OUT_C])
                    stage = stgp.tile([OUT_C, SG * GCOLS], mybir.dt.bfloat16,
                                      tag="stage")
                    for gl in range(ngr):
                        g = g0 + gl
                        S = stp.tile([128, GCH * W], mybir.dt.float8e4,
                                     tag="S")
                        iota_bc = (iota_t[:].unsqueeze(1)
                                   .broadcast_to([128, GCH, W]))
                        slot_bc = (slot_t[:, g * GCH:(g + 1) * GCH]
                                   .unsqueeze(2).broadcast_to([128, GCH, W]))
                        nc.vector.scalar_tensor_tensor(
                            S[:], iota_bc, 1.0, slot_bc,
                            mybir.AluOpType.mult, mybir.AluOpType.is_equal)

                        agg = pagg.tile([OUT_C, GCOLS], mybir.dt.float32,
                                        tag="agg")
                        for b in range(GRP):
                            kk = gl * GCH + 2 * b
                            lhs = (mt[:, kk * OUT_C:(kk + 2) * OUT_C]
                                   .rearrange("p (two f) -> p two f", two=2))
                            rhs = (S[:, 2 * b * W:(2 * b + 2) * W]
                                   .rearrange("p (two f) -> p two f", two=2))
                            nc.tensor.matmul(
                                agg[:, b * W:(b + 1) * W], lhs, rhs,
                                start=True, stop=True,
                                perf_mode=mybir.MatmulPerfMode.DoubleRow)
                        nc.scalar.activation(
                            stage[:, gl * GCOLS:(gl + 1) * GCOLS],
                            agg[:], Ident, bias=b4_t[:],
                            scale=1.0 / MSG_SCALE)
                    nc.scalar.dma_start(
                        ost_d.ap()[:, g0 * GCOLS:g0 * GCOLS + ngr * GCOLS],
                        stage[:, :ngr * GCOLS])

            if loop_reps:
                with tc.For_i(0, loop_reps, 1):
                    body()
            else:
                for _ in range(reps):
                    body()
    nc.compile()
    return nc


# ----------------------------------------------------------------------------
# full kernel
# ----------------------------------------------------------------------------

LAST_HW_EXEC_NS = None
LAST_NB = None


def _run(nc, in_maps):
    _import_bass()
    res = bass_utils.run_bass_kernel_spmd(nc, in_maps,
                                          core_ids=list(range(N_CORES)))
    return res.results


def kernel(x, edge_index, W3, b3, W4, b4):
    global LAST_NB
    _import_bass()
    x = np.asarray(x, np.float32)
    prep = preprocess(x, np.asarray(edge_index))
    NB, NCH = prep["NB"], prep["NCH"]
    LAST_NB = NB
    COLS = NB * W

    iota_np = np.tile(np.arange(W, dtype=np.float32), (128, 1)).astype(bf16)
    W3_bf = np.asarray(W3, np.float32).astype(bf16)
    W4_bf = np.asarray(W4, np.float32).astype(bf16)
    b3_c = np.asarray(b3, np.float32).reshape(HID_C, 1)
    b4_c = np.asarray(b4, np.float32).reshape(OUT_C, 1)

    nc1 = build_layer1(NB)
    in1 = []
    for c in prep["cores"]:
        in1.append(dict(
            msg=_stage_msgs(x, c["src_t"], c["e_norm"]),
            slot=c["meta_slot"], iota=iota_np,
            w3=W3_bf, b3c=b3_c, w4=W4_bf))
    res1 = _run(nc1, in1)
    # z, transposed per core: [OUT_C, COLS] -> all cores' columns concatenated
    zT = np.concatenate([np.asarray(r["zstage"]) for r in res1], axis=1)
    z = np.ascontiguousarray(zT.T)            # [8*COLS, OUT_C] bf16

    nc2 = build_layer2(NB)
    in2 = []
    for c in prep["cores"]:
        in2.append(dict(
            msg=_stage_msgs(z, c["g2_ind"], c["e_norm"]),
            slot=c["meta_slot"], iota=iota_np, b4c=b4_c))
    res2 = _run(nc2, in2)
    oT = np.concatenate([np.asarray(r["ostage"]) for r in res2], axis=1)

    sr = prep["stage_row"]
    out = np.ascontiguousarray(oT.T)[sr]
    return out.astype(np.float32)
